# revision 13
# baseline (speedup 1.0000x reference)
"""MultiHeadAttention Trainium2 kernel.

Problem (hardcoded): S=2048, B=2, D=1024, H=16, HD=64, fp32 I/O.
  q = query @ w_q.T + b_q   (same for k, v), heads split from D
  scores[i,j,b,h] = (q_i . k_j)/8, masked where mask[j]==0, softmax over j
  out[i,b,:] = concat_h( sum_j p_ij v_j )

Sharding: 8 cores = 2 batches x 4 head-groups (4 heads / 256 dims each).
Host-side prep: cast to bf16, transpose to [D, seq] layout, and compact the
key/value sequence to the unmasked positions only (masked j contribute 0),
padded to a multiple of 128.

Per-core program (Tile framework):
  - Q,K projections: out qT/kT[o, i] with o (head dims) on partitions.
  - V projection flipped (x^T tiles stationary) giving V[j, o] with j on
    partitions; extended with a ones column (zeroed on padding) so the
    softmax denominator falls out of the PV matmul.
  - Scores computed transposed: S^T[j, i] = kT.T @ qT, two heads packed in
    the 128-row PE array via base-partition row tiling (d=64 each).
  - P^T = exp(S^T / 8) on ACT (bf16 out), PV with P^T tiles stationary:
    out[i, vd] += P^T[j, i-tile].T @ Vext[j, :], fp32 PSUM accumulation.
  - normalize rows by reciprocal of the ones-column sum, DMA out.
"""

import math
import os
import sys

sys.path.insert(0, "/opt/trn_rl_repo")

import numpy as np
import ml_dtypes

import concourse.bass as bass
import concourse.tile as tile
from concourse import bacc, mybir
from concourse.bass_utils import run_bass_kernel_spmd

S, B, D, H, HD = 2048, 2, 1024, 16, 64
N_CORES = 8
GROUPS = 4          # head groups (cores per batch)
GH = H // GROUPS    # heads per core = 4
GD = GH * HD        # dims per core = 256
KT = D // 128       # contraction k-tiles = 8
NIT = S // 128      # i tiles = 16
IBLK = 1024         # i block (exp granularity / P^T tile width)
NIB = S // IBLK     # i blocks = 2

BF16 = mybir.dt.bfloat16
F32 = mybir.dt.float32

_CACHE = {}


def _chunks(total, step):
    out = []
    o = 0
    while o < total:
        n = min(step, total - o)
        out.append((o, n))
        o += n
    return out


def _build(J, J_real, use_bias):
    """Build + compile the per-core Bass program (identical on all cores)."""
    NJT = J // 128
    nc = bacc.Bacc("TRN2", target_bir_lowering=False, debug=False,
                   enable_asserts=False)

    xq_d = nc.dram_tensor("xq", (D, S), BF16, kind="ExternalInput")
    xk_d = nc.dram_tensor("xk", (D, J), BF16, kind="ExternalInput")
    xv_d = nc.dram_tensor("xv", (D, J), BF16, kind="ExternalInput")
    wq_d = nc.dram_tensor("wq", (D, GD), BF16, kind="ExternalInput")
    wk_d = nc.dram_tensor("wk", (D, GD), BF16, kind="ExternalInput")
    wv_d = nc.dram_tensor("wv", (D, GD), BF16, kind="ExternalInput")
    mpad_d = nc.dram_tensor("mpad", (J, 1), BF16, kind="ExternalInput")
    if use_bias:
        bq_d = nc.dram_tensor("bq", (GD,), F32, kind="ExternalInput")
        bk_d = nc.dram_tensor("bk", (GD,), F32, kind="ExternalInput")
        bv_d = nc.dram_tensor("bv", (GD,), F32, kind="ExternalInput")
    out_d = nc.dram_tensor("out", (S, GD), F32, kind="ExternalOutput")

    # SBUF budget for the P^T pool: aim for 4*NJT bufs (both head-pairs fully
    # double-buffered), shrink when J is large.
    pt_tile_bytes = 128 * IBLK * 2
    fixed = (KT * 128 * S * 2              # xq tiles
             + 2 * KT * 128 * J * 2        # xk, xv tiles
             + 3 * KT * 128 * GD * 2       # weights
             + 4 * 128 * S * 2             # qT/kT pool
             + NJT * 128 * (GH * (HD + 1) + 4) * 2   # vext
             + 16 * 128 * GD * 4           # out staging
             + 3 * 1024 * 1024)            # slack (const pool etc.)
    budget = 23 * 1024 * 1024 - fixed
    pt_bufs = min(4 * NJT, max(2 * NJT, budget // pt_tile_bytes))

    VW = GH * (HD + 1)  # vext width: per head 64 v-cols + 1 ones-col

    with tile.TileContext(nc) as tc:
        with (
            tc.tile_pool(name="xq", bufs=KT) as xq_p,
            tc.tile_pool(name="xk", bufs=KT) as xk_p,
            tc.tile_pool(name="xv", bufs=KT) as xv_p,
            tc.tile_pool(name="w", bufs=3 * KT) as w_p,
            tc.tile_pool(name="qk", bufs=4) as qk_p,
            tc.tile_pool(name="vext", bufs=NJT) as vext_p,
            tc.tile_pool(name="pt", bufs=pt_bufs) as pt_p,
            tc.tile_pool(name="stage", bufs=16) as stage_p,
            tc.tile_pool(name="small", bufs=48) as small_p,
            tc.tile_pool(name="psum", bufs=4, space="PSUM") as ps_p,
        ):
            # ---- input loads (consumed lazily by Tile scheduling) ----
            xq_t = []
            xk_t = []
            xv_t = []
            wq_t = []
            wk_t = []
            wv_t = []
            for k in range(KT):
                t = xq_p.tile([128, S], BF16, tag="xq")
                nc.sync.dma_start(t[:], xq_d.ap()[k * 128:(k + 1) * 128, :])
                xq_t.append(t)
                t = xk_p.tile([128, J], BF16, tag="xk")
                nc.sync.dma_start(t[:], xk_d.ap()[k * 128:(k + 1) * 128, :])
                xk_t.append(t)
                t = xv_p.tile([128, J], BF16, tag="xv")
                nc.sync.dma_start(t[:], xv_d.ap()[k * 128:(k + 1) * 128, :])
                xv_t.append(t)
                for w_d, lst in ((wq_d, wq_t), (wk_d, wk_t), (wv_d, wv_t)):
                    t = w_p.tile([128, GD], BF16, tag="w")
                    nc.sync.dma_start(t[:], w_d.ap()[k * 128:(k + 1) * 128, :])
                    lst.append(t)

            if use_bias:
                bq_c = []
                bk_c = []
                for ot in range(2):
                    t = small_p.tile([128, 1], F32, tag="bias")
                    nc.sync.dma_start(t[:], bq_d.ap()[ot * 128:(ot + 1) * 128])
                    bq_c.append(t)
                    t = small_p.tile([128, 1], F32, tag="bias")
                    nc.sync.dma_start(t[:], bk_d.ap()[ot * 128:(ot + 1) * 128])
                    bk_c.append(t)
                bv_row = small_p.tile([1, GD], F32, tag="bvrow")
                nc.sync.dma_start(bv_row[:], bv_d.ap()[None, :])
                ones_row = small_p.tile([1, 128], BF16, tag="ones")
                nc.vector.memset(ones_row[:], 1.0)

            # ---- projections ----
            qT = []   # per otile: [128, S] bf16  (o on partitions)
            kTt = []  # per otile: [128, J] bf16
            scale = 1.0 / math.sqrt(HD)  # 0.125, folded into the exp

            def proj_qk(x_tiles, w_tiles, dst_list, bias_cols, width, ot):
                dst = qk_p.tile([128, S], BF16, tag="qk")
                for (o, n) in _chunks(width, 512):
                    ps = ps_p.tile([128, 512], F32, tag="ps")
                    for k in range(KT):
                        nc.tensor.matmul(
                            ps[:, 0:n],
                            lhsT=w_tiles[k][:, ot * 128:(ot + 1) * 128],
                            rhs=x_tiles[k][:, o:o + n],
                            start=(k == 0), stop=(k == KT - 1))
                    if use_bias:
                        nc.vector.tensor_scalar_add(
                            dst[:, o:o + n], ps[:, 0:n], bias_cols[ot])
                    else:
                        nc.vector.tensor_copy(dst[:, o:o + n], ps[:, 0:n])
                dst_list.append(dst)

            # K then Q for head-pair 0 first so scores can start early.
            if use_bias:
                proj_qk(xk_t, wk_t, kTt, bk_c, J, 0)
                proj_qk(xq_t, wq_t, qT, bq_c, S, 0)
            else:
                proj_qk(xk_t, wk_t, kTt, None, J, 0)
                proj_qk(xq_t, wq_t, qT, None, S, 0)

            # V projection (flipped): V[j, o] with j on partitions.
            vext = []
            pad = J - J_real
            for jt in range(NJT):
                ps = ps_p.tile([128, GD], F32, tag="ps")
                for k in range(KT):
                    nc.tensor.matmul(
                        ps[:, :],
                        lhsT=xv_t[k][:, jt * 128:(jt + 1) * 128],
                        rhs=wv_t[k][:, :],
                        start=(k == 0), stop=(k == KT - 1) and not use_bias)
                if use_bias:
                    nc.tensor.matmul(ps[:, :], lhsT=ones_row[:, :],
                                     rhs=bv_row[:, :], start=False, stop=True)
                ve = vext_p.tile([128, VW], BF16, tag="vext")
                for h in range(GH):
                    nc.vector.tensor_copy(
                        ve[:, h * (HD + 1):h * (HD + 1) + HD],
                        ps[:, h * HD:(h + 1) * HD])
                    # ones column (0 on padding rows) -> softmax denominator
                    nc.sync.dma_start(
                        ve[:, h * (HD + 1) + HD:h * (HD + 1) + HD + 1],
                        mpad_d.ap()[jt * 128:(jt + 1) * 128, :])
                vext.append(ve)

            # remaining projections (head-pair 1)
            if use_bias:
                proj_qk(xq_t, wq_t, qT, bq_c, S, 1)
                proj_qk(xk_t, wk_t, kTt, bk_c, J, 1)
            else:
                proj_qk(xq_t, wq_t, qT, None, S, 1)
                proj_qk(xk_t, wk_t, kTt, None, J, 1)

            # ---- attention, blocked over i ----
            stage_t = {}

            for ib in range(NIB):
                i0 = ib * IBLK
                for hp in range(2):
                    pt = {}
                    # scores + exp for this head pair of this i-block
                    for jt in range(NJT):
                        psA = ps_p.tile([128, IBLK], F32, tag="ps")
                        psB = ps_p.tile([128, IBLK], F32, tag="ps")
                        for (o, n) in _chunks(IBLK, 512):
                            nc.tensor.matmul(
                                psA[:, o:o + n],
                                lhsT=kTt[hp][0:64, jt * 128:(jt + 1) * 128],
                                rhs=qT[hp][0:64, i0 + o:i0 + o + n],
                                start=True, stop=True)
                            nc.tensor.matmul(
                                psB[:, o:o + n],
                                lhsT=kTt[hp][64:128, jt * 128:(jt + 1) * 128],
                                rhs=qT[hp][64:128, i0 + o:i0 + o + n],
                                start=True, stop=True)
                        ptA = pt_p.tile([128, IBLK], BF16, tag="pt")
                        ptB = pt_p.tile([128, IBLK], BF16, tag="pt")
                        nc.scalar.activation(ptA[:], psA[:],
                                             mybir.ActivationFunctionType.Exp,
                                             scale=scale)
                        nc.scalar.activation(ptB[:], psB[:],
                                             mybir.ActivationFunctionType.Exp,
                                             scale=scale)
                        pt[(hp * 2, jt)] = ptA
                        pt[(hp * 2 + 1, jt)] = ptB

                    # PV + normalize for this head pair of this i-block
                    for itl in range(IBLK // 128):
                        it = ib * (IBLK // 128) + itl
                        if hp == 0 and it not in stage_t:
                            stage_t[it] = stage_p.tile([128, GD], F32,
                                                       tag="stage",
                                                       name=f"stage{it}")
                        st = stage_t[it]
                        for hl in range(2):
                            h = hp * 2 + hl
                            pv = ps_p.tile([128, HD + 1], F32, tag="ps")
                            for jt in range(NJT):
                                nc.tensor.matmul(
                                    pv[:, :],
                                    lhsT=pt[(h, jt)][:, itl * 128:
                                                     (itl + 1) * 128],
                                    rhs=vext[jt][:, h * (HD + 1):
                                                 (h + 1) * (HD + 1)],
                                    start=(jt == 0), stop=(jt == NJT - 1))
                            rc = small_p.tile([128, 1], F32, tag="recip")
                            nc.vector.reciprocal(rc[:], pv[:, HD:HD + 1])
                            nc.vector.tensor_scalar_mul(
                                st[:, h * HD:(h + 1) * HD], pv[:, 0:HD],
                                rc[:])
                        if hp == 1:
                            nc.sync.dma_start(
                                out_d.ap()[it * 128:(it + 1) * 128, :],
                                st[:])

    nc.compile()
    return nc


def _prep_and_run(inputs, trace=False):
    query = np.asarray(inputs["query"], dtype=np.float32)
    key = np.asarray(inputs["key"], dtype=np.float32)
    value = np.asarray(inputs["value"], dtype=np.float32)
    mask = np.asarray(inputs["mask"]).reshape(S)
    w_q = np.asarray(inputs["w_q"], dtype=np.float32)
    b_q = np.asarray(inputs["b_q"], dtype=np.float32)
    w_k = np.asarray(inputs["w_k"], dtype=np.float32)
    b_k = np.asarray(inputs["b_k"], dtype=np.float32)
    w_v = np.asarray(inputs["w_v"], dtype=np.float32)
    b_v = np.asarray(inputs["b_v"], dtype=np.float32)

    use_bias = bool(np.any(b_q) or np.any(b_k) or np.any(b_v))

    # compact key/value over masked-out positions
    idx = np.nonzero(mask != 0)[0]
    J_real = int(len(idx))
    assert J_real > 0, "all positions masked: softmax undefined"
    J = max(128, ((J_real + 127) // 128) * 128)
    key_c = np.zeros((J, B, D), np.float32)
    key_c[:J_real] = key[idx]
    value_c = np.zeros((J, B, D), np.float32)
    value_c[:J_real] = value[idx]

    bf = ml_dtypes.bfloat16
    mpad = np.zeros((J, 1), bf)
    mpad[:J_real] = 1
    in_maps = []
    for core in range(N_CORES):
        b = core // GROUPS
        g = core % GROUPS
        hs = slice(g * GD, (g + 1) * GD)
        m = {
            "xq": np.ascontiguousarray(query[:, b, :].T).astype(bf),
            "xk": np.ascontiguousarray(key_c[:, b, :].T).astype(bf),
            "xv": np.ascontiguousarray(value_c[:, b, :].T).astype(bf),
            "wq": np.ascontiguousarray(w_q[hs, :].T).astype(bf),
            "wk": np.ascontiguousarray(w_k[hs, :].T).astype(bf),
            "wv": np.ascontiguousarray(w_v[hs, :].T).astype(bf),
            "mpad": mpad,
        }
        if use_bias:
            m["bq"] = np.ascontiguousarray(b_q[hs])
            m["bk"] = np.ascontiguousarray(b_k[hs])
            m["bv"] = np.ascontiguousarray(b_v[hs])
        in_maps.append(m)

    ck = (J, J_real, use_bias)
    if ck not in _CACHE:
        _CACHE[ck] = _build(J, J_real, use_bias)
    nc = _CACHE[ck]

    kwargs = {}
    if trace:
        kwargs = dict(trace=True, trace_cores=list(range(N_CORES)))
    res = run_bass_kernel_spmd(nc, in_maps, core_ids=list(range(N_CORES)),
                               **kwargs)

    out = np.empty((S, B, D), np.float32)
    for core in range(N_CORES):
        b = core // GROUPS
        g = core % GROUPS
        out[:, b, g * GD:(g + 1) * GD] = res.results[core]["out"]
    return out, res


def kernel(**inputs):
    out, _ = _prep_and_run(inputs, trace=False)
    return out


def run_traced(**inputs):
    _, res = _prep_and_run(inputs, trace=True)
    return res
